# revision 1
# baseline (speedup 1.0000x reference)
"""GaussianMLP sampling kernel for 8 trn2 NeuronCores (pure data parallel).

reference:
    h      = relu(x @ W_emb + b_emb)        x:[B,128] W_emb:[128,256]
    mean   = h @ W_mean + b_mean            W_mean:[256,128]
    logvar = h @ W_logvar + b_logvar        W_logvar:[256,128]
    z      = mean + exp(0.5*logvar) * eps
    returns (z, mean, logvar)

Sharding: x/eps split along batch across 8 cores; weights replicated.

Per-core dataflow (ROWS_PER_TILE=512 rows/iteration):
  - DMA x tile [128p, 4, 128] (natural layout, partition=row)
  - PE transpose 4x [128,128] -> xT [d_in=128p, 512 rows] in PSUM, copy to SBUF
  - hT = W_emb.T @ x.T via 2 matmuls (lhsT=W_emb chunk, rhs=xT) -> PSUM
  - ACT relu(hT + b_emb) PSUM->SBUF (per-partition bias)
  - mean/logvar: bias seeded by a K=1 ones-matmul over the whole PSUM bank,
    then per-128-row subtile: accumulate hT0.T@Wm0 + hT1.T@Wm1
  - epilogue: ACT copies logvar out + exp(0.5*logvar); DVE mean copy,
    se=std*eps, z=mean+se; DMA 3 outputs
"""

import os
import sys

sys.path.insert(0, "/opt/trn_rl_repo")

import numpy as np

from contextlib import ExitStack

from concourse import bacc, bass, masks, mybir, tile
from concourse.alu_op_type import AluOpType
from concourse.bass_utils import run_bass_kernel_spmd

N_CORES = 8
B = 524288
D_IN = 128
D_H = 256
D_OUT = 128
ROWS_PER_CORE = B // N_CORES  # 65536
ROWS_PER_TILE = 512
N_TILES = ROWS_PER_CORE // ROWS_PER_TILE  # 128
S = ROWS_PER_TILE // 128  # 4 subtiles of 128 rows

F32 = mybir.dt.float32
# dtype for the two matmul layers (float32 = exact-ish, bfloat16 = faster PE)
BF16 = mybir.dt.bfloat16
L1_DT = BF16
L2_DT = BF16


def build_bass(rows_per_core=ROWS_PER_CORE):
    nc = bacc.Bacc("TRN2", target_bir_lowering=False, debug=False)
    n_tiles = rows_per_core // ROWS_PER_TILE

    x_ext = nc.declare_dram_parameter("x", [rows_per_core, D_IN], F32, isOutput=False)
    eps_ext = nc.declare_dram_parameter(
        "eps", [rows_per_core, D_OUT], F32, isOutput=False
    )
    We_ext = nc.declare_dram_parameter("W_emb", [D_IN, D_H], F32, isOutput=False)
    be_ext = nc.declare_dram_parameter("b_emb", [D_H], F32, isOutput=False)
    Wm_ext = nc.declare_dram_parameter("W_mean", [D_H, D_OUT], F32, isOutput=False)
    bm_ext = nc.declare_dram_parameter("b_mean", [D_OUT], F32, isOutput=False)
    Wl_ext = nc.declare_dram_parameter("W_logvar", [D_H, D_OUT], F32, isOutput=False)
    bl_ext = nc.declare_dram_parameter("b_logvar", [D_OUT], F32, isOutput=False)
    z_ext = nc.declare_dram_parameter("z", [rows_per_core, D_OUT], F32, isOutput=True)
    mean_ext = nc.declare_dram_parameter(
        "mean", [rows_per_core, D_OUT], F32, isOutput=True
    )
    lv_ext = nc.declare_dram_parameter(
        "logvar", [rows_per_core, D_OUT], F32, isOutput=True
    )

    # tiled DRAM views: row = t*ROWS_PER_TILE + s*128 + p
    xv = x_ext.rearrange("(t s p) d -> t p s d", s=S, p=128)
    ev = eps_ext.rearrange("(t s p) d -> t p s d", s=S, p=128)
    zv = z_ext.rearrange("(t s p) d -> t p s d", s=S, p=128)
    mv = mean_ext.rearrange("(t s p) d -> t p s d", s=S, p=128)
    lvv = lv_ext.rearrange("(t s p) d -> t p s d", s=S, p=128)

    with tile.TileContext(nc) as tc, ExitStack() as ctx:
        const = ctx.enter_context(tc.tile_pool(name="const", bufs=1))
        xin = ctx.enter_context(tc.tile_pool(name="xin", bufs=3))
        epool = ctx.enter_context(tc.tile_pool(name="eps", bufs=3))
        xTp = ctx.enter_context(tc.tile_pool(name="xT", bufs=3))
        hTp = ctx.enter_context(tc.tile_pool(name="hTs", bufs=2))
        outs = ctx.enter_context(tc.tile_pool(name="outs", bufs=3))
        psA = ctx.enter_context(tc.tile_pool(name="psA", bufs=2, space="PSUM"))
        psB = ctx.enter_context(tc.tile_pool(name="psB", bufs=1, space="PSUM"))
        psC = ctx.enter_context(tc.tile_pool(name="psC", bufs=2, space="PSUM"))

        # --- constants / weights (loaded once) ---
        ident = const.tile([128, 128], F32)
        masks.make_identity(nc, ident[:])

        We_sb = const.tile([128, D_H], L1_DT)
        dma_w = nc.gpsimd if L1_DT != F32 else nc.sync
        dma_w.dma_start(We_sb[:], We_ext[:])

        Wm_sb = const.tile([128, 2, D_OUT], L2_DT)
        Wl_sb = const.tile([128, 2, D_OUT], L2_DT)
        dma_w2 = nc.gpsimd if L2_DT != F32 else nc.sync
        dma_w2.dma_start(Wm_sb[:], Wm_ext.rearrange("(c p) d -> p c d", p=128))
        dma_w2.dma_start(Wl_sb[:], Wl_ext.rearrange("(c p) d -> p c d", p=128))

        be_sb = const.tile([128, 2], F32)
        nc.sync.dma_start(be_sb[:], be_ext.rearrange("(c p) -> p c", p=128))

        ones_sb = const.tile([1, 128], F32)
        nc.vector.memset(ones_sb[:], 1.0)
        bm_rep = const.tile([1, S * D_OUT], F32)
        bl_rep = const.tile([1, S * D_OUT], F32)
        for s in range(S):
            nc.sync.dma_start(
                bm_rep[0:1, s * D_OUT : (s + 1) * D_OUT],
                bm_ext.rearrange("(o d) -> o d", o=1),
            )
            nc.sync.dma_start(
                bl_rep[0:1, s * D_OUT : (s + 1) * D_OUT],
                bl_ext.rearrange("(o d) -> o d", o=1),
            )

        for t in range(n_tiles):
            x_sb = xin.tile([128, S, D_IN], F32, tag="x")
            nc.sync.dma_start(x_sb[:], xv[t])
            eps_sb = epool.tile([128, S, D_OUT], F32, tag="eps")
            nc.sync.dma_start(eps_sb[:], ev[t])

            # transpose x -> xT [d_in, rows]
            xT_ps = psA.tile([128, ROWS_PER_TILE], F32, tag="xT")
            for s in range(S):
                nc.tensor.transpose(
                    xT_ps[:, s * 128 : (s + 1) * 128], x_sb[:, s, :], ident[:]
                )
            xT_sb = xTp.tile([128, ROWS_PER_TILE], L1_DT, tag="xTs")
            nc.vector.tensor_copy(xT_sb[:], xT_ps[:])

            # layer 1: hT[c] = W_emb[:, c].T @ xT  (c: two 128-wide d_h chunks)
            hT_ps0 = psB.tile([128, ROWS_PER_TILE], F32, tag="hT0")
            hT_ps1 = psB.tile([128, ROWS_PER_TILE], F32, tag="hT1")
            nc.tensor.matmul(
                hT_ps0[:], We_sb[:, 0:128], xT_sb[:], start=True, stop=True
            )
            nc.tensor.matmul(
                hT_ps1[:], We_sb[:, 128:256], xT_sb[:], start=True, stop=True
            )
            hT_sb0 = hTp.tile([128, ROWS_PER_TILE], L2_DT, tag="h0")
            hT_sb1 = hTp.tile([128, ROWS_PER_TILE], L2_DT, tag="h1")
            nc.scalar.activation(
                hT_sb0[:],
                hT_ps0[:],
                mybir.ActivationFunctionType.Relu,
                bias=be_sb[:, 0:1],
            )
            nc.scalar.activation(
                hT_sb1[:],
                hT_ps1[:],
                mybir.ActivationFunctionType.Relu,
                bias=be_sb[:, 1:2],
            )

            # layer 2: mean/logvar [rows, d_out] per 128-row subtile,
            # bias seeded across the whole 512-wide bank by a K=1 matmul
            mean_ps = psC.tile([128, S * D_OUT], F32, tag="mean")
            lv_ps = psC.tile([128, S * D_OUT], F32, tag="lv")
            nc.tensor.matmul(
                mean_ps[:], ones_sb[:], bm_rep[:],
                start=True, stop=False, skip_group_check=True,
            )
            nc.tensor.matmul(
                lv_ps[:], ones_sb[:], bl_rep[:],
                start=True, stop=False, skip_group_check=True,
            )
            for s in range(S):
                sl = slice(s * 128, (s + 1) * 128)
                so = slice(s * D_OUT, (s + 1) * D_OUT)
                nc.tensor.matmul(
                    mean_ps[:, so], hT_sb0[:, sl], Wm_sb[:, 0, :],
                    start=False, stop=False, skip_group_check=True,
                )
                nc.tensor.matmul(
                    mean_ps[:, so], hT_sb1[:, sl], Wm_sb[:, 1, :],
                    start=False, stop=(s == S - 1), skip_group_check=True,
                )
                nc.tensor.matmul(
                    lv_ps[:, so], hT_sb0[:, sl], Wl_sb[:, 0, :],
                    start=False, stop=False, skip_group_check=True,
                )
                nc.tensor.matmul(
                    lv_ps[:, so], hT_sb1[:, sl], Wl_sb[:, 1, :],
                    start=False, stop=(s == S - 1), skip_group_check=True,
                )

            # epilogue
            lv_sb = outs.tile([128, S * D_OUT], F32, tag="lvs")
            nc.scalar.activation(
                lv_sb[:], lv_ps[:], mybir.ActivationFunctionType.Copy
            )
            std_sb = outs.tile([128, S * D_OUT], F32, tag="std")
            nc.scalar.activation(
                std_sb[:], lv_ps[:], mybir.ActivationFunctionType.Exp, scale=0.5
            )
            mean_sb = outs.tile([128, S * D_OUT], F32, tag="means")
            nc.vector.tensor_copy(mean_sb[:], mean_ps[:])
            se_sb = outs.tile([128, S * D_OUT], F32, tag="se")
            nc.vector.tensor_mul(
                se_sb[:], std_sb[:], eps_sb[:].rearrange("p s d -> p (s d)")
            )
            z_sb = outs.tile([128, S * D_OUT], F32, tag="z")
            nc.vector.scalar_tensor_tensor(
                z_sb[:], mean_ps[:], 1.0, se_sb[:], AluOpType.mult, AluOpType.add
            )

            nc.sync.dma_start(zv[t], z_sb[:].rearrange("p (s d) -> p s d", s=S))
            nc.sync.dma_start(mv[t], mean_sb[:].rearrange("p (s d) -> p s d", s=S))
            nc.sync.dma_start(lvv[t], lv_sb[:].rearrange("p (s d) -> p s d", s=S))

    nc.finalize()
    return nc


_NC_CACHE = None


def _get_nc():
    global _NC_CACHE
    if _NC_CACHE is None:
        _NC_CACHE = build_bass()
    return _NC_CACHE


def _run(inputs, trace=False, **kw):
    nc = _get_nc()
    xs = np.ascontiguousarray(np.asarray(inputs["x"], dtype=np.float32))
    es = np.ascontiguousarray(np.asarray(inputs["eps"], dtype=np.float32))
    weights = {
        k: np.ascontiguousarray(np.asarray(inputs[k], dtype=np.float32))
        for k in ("W_emb", "b_emb", "W_mean", "b_mean", "W_logvar", "b_logvar")
    }
    in_maps = []
    for c in range(N_CORES):
        sl = slice(c * ROWS_PER_CORE, (c + 1) * ROWS_PER_CORE)
        in_maps.append({"x": xs[sl], "eps": es[sl], **weights})
    res = run_bass_kernel_spmd(nc, in_maps, list(range(N_CORES)), trace=trace, **kw)
    z = np.concatenate([res.results[c]["z"] for c in range(N_CORES)], axis=0)
    mean = np.concatenate([res.results[c]["mean"] for c in range(N_CORES)], axis=0)
    lv = np.concatenate([res.results[c]["logvar"] for c in range(N_CORES)], axis=0)
    return (z, mean, lv), res


def kernel(**inputs):
    out, _ = _run(inputs, trace=False)
    return out


if __name__ == "__main__":
    rng = np.random.default_rng(0)
    demo = {
        "x": rng.standard_normal((B, D_IN), dtype=np.float32),
        "eps": rng.standard_normal((B, D_OUT), dtype=np.float32),
        "W_emb": rng.standard_normal((D_IN, D_H), dtype=np.float32) * 0.088,
        "b_emb": rng.standard_normal((D_H,), dtype=np.float32) * 0.05,
        "W_mean": rng.standard_normal((D_H, D_OUT), dtype=np.float32) * 0.06,
        "b_mean": rng.standard_normal((D_OUT,), dtype=np.float32) * 0.03,
        "W_logvar": rng.standard_normal((D_H, D_OUT), dtype=np.float32) * 0.06,
        "b_logvar": rng.standard_normal((D_OUT,), dtype=np.float32) * 0.03,
    }
    z, m, l = kernel(**demo)
    print("shapes", z.shape, m.shape, l.shape)



# revision 7
# speedup vs baseline: 1.9079x; 1.9079x over previous
"""GaussianMLP sampling kernel for 8 trn2 NeuronCores (pure data parallel).

reference:
    h      = relu(x @ W_emb + b_emb)        x:[B,128] W_emb:[128,256]
    mean   = h @ W_mean + b_mean            W_mean:[256,128]
    logvar = h @ W_logvar + b_logvar        W_logvar:[256,128]
    z      = mean + exp(0.5*logvar) * eps
    returns (z, mean, logvar)

Sharding: x/eps split along batch across 8 cores; weights replicated.

v2 design (memory-regime):
  - All bulk I/O in bf16 (host converts): halves HBM traffic; outputs are
    packed into one [3, R, 128] DRAM tensor (z/mean/logvar) so each
    512-row chunk needs a single output DMA.
  - DRAM views "(t p s) d -> t p (s d)" keep per-partition contiguous
    runs >= 1 KiB so the DMA engines run at full rate.
  - PE does only the irreducible work per 512-row chunk: 4 bf16
    transposes (512 cyc) + L1 (1024 cyc) + L2 (2048 cyc). No fp32 bias
    seed matmuls: L1 bias rides the ACT relu (per-partition bias);
    L2 biases are added by DVE from precomputed broadcast tiles.
  - Software-pipelined loop (3 stages) so transposes of chunk c overlap
    the matmuls of c-1 and the epilogue of c-2.
"""

import sys

sys.path.insert(0, "/opt/trn_rl_repo")

import numpy as np
import ml_dtypes

from contextlib import ExitStack

from concourse import bacc, bass, masks, mybir, tile
from concourse.alu_op_type import AluOpType
from concourse.bass_utils import run_bass_kernel_spmd

N_CORES = 8
B = 524288
D_IN = 128
D_H = 256
D_OUT = 128
ROWS_PER_CORE = B // N_CORES  # 65536

S_DMA = 16  # rows-per-partition per input DMA tile (2048 rows)
CHUNK_S = 4  # 512-row compute chunk = 4 x 128-row subtiles
CHUNK = CHUNK_S * 128
CHUNKS_PER_TILE = S_DMA // CHUNK_S  # 4
TILE_ROWS = 128 * S_DMA  # 2048

F32 = mybir.dt.float32
BF16 = mybir.dt.bfloat16
NPBF16 = ml_dtypes.bfloat16


def build_bass(rows_per_core=ROWS_PER_CORE):
    nc = bacc.Bacc("TRN2", target_bir_lowering=False, debug=False)
    n_tiles = rows_per_core // TILE_ROWS
    n_chunks = rows_per_core // CHUNK

    x_ext = nc.declare_dram_parameter("x", [rows_per_core, D_IN], BF16, isOutput=False)
    eps_ext = nc.declare_dram_parameter(
        "eps", [rows_per_core, D_OUT], BF16, isOutput=False
    )
    We_ext = nc.declare_dram_parameter("W_emb", [D_IN, D_H], F32, isOutput=False)
    be_ext = nc.declare_dram_parameter("b_emb", [D_H], F32, isOutput=False)
    Wm_ext = nc.declare_dram_parameter("W_mean", [D_H, D_OUT], F32, isOutput=False)
    bm_ext = nc.declare_dram_parameter("b_mean", [D_OUT], F32, isOutput=False)
    Wl_ext = nc.declare_dram_parameter("W_logvar", [D_H, D_OUT], F32, isOutput=False)
    bl_ext = nc.declare_dram_parameter("b_logvar", [D_OUT], F32, isOutput=False)
    out_ext = nc.declare_dram_parameter(
        "out", [3, rows_per_core, D_OUT], BF16, isOutput=True
    )

    # row = t*TILE_ROWS + p*S_DMA + s ; per-partition contiguous run = s*d
    xv = x_ext.rearrange("(t p s) d -> t p (s d)", p=128, s=S_DMA)
    ev = eps_ext.rearrange("(t p s) d -> t p (s d)", p=128, s=S_DMA)
    # output view sliced per 512-row chunk: [p, 3, 4, 128]
    ov = out_ext.rearrange("c (t p s) d -> t p c s d", p=128, s=S_DMA)

    with tile.TileContext(nc) as tc, ExitStack() as ctx:
        const = ctx.enter_context(tc.tile_pool(name="const", bufs=1))
        xin = ctx.enter_context(tc.tile_pool(name="xin", bufs=3))
        epool = ctx.enter_context(tc.tile_pool(name="eps", bufs=3))
        xTp = ctx.enter_context(tc.tile_pool(name="xT", bufs=2))
        hTp = ctx.enter_context(tc.tile_pool(name="hTs", bufs=2))
        stdp = ctx.enter_context(tc.tile_pool(name="std", bufs=2))
        sep = ctx.enter_context(tc.tile_pool(name="se", bufs=2))
        outs = ctx.enter_context(tc.tile_pool(name="outs", bufs=3))
        psA = ctx.enter_context(tc.tile_pool(name="psA", bufs=2, space="PSUM"))
        psB = ctx.enter_context(tc.tile_pool(name="psB", bufs=1, space="PSUM"))
        psC = ctx.enter_context(tc.tile_pool(name="psC", bufs=2, space="PSUM"))

        # --- constants / weights (loaded once) ---
        ident = const.tile([128, 128], BF16)
        masks.make_identity(nc, ident[:])

        We_sb = const.tile([128, D_H], BF16)
        nc.gpsimd.dma_start(We_sb[:], We_ext[:])
        Wm_sb = const.tile([128, 2, D_OUT], BF16)
        Wl_sb = const.tile([128, 2, D_OUT], BF16)
        nc.gpsimd.dma_start(Wm_sb[:], Wm_ext.rearrange("(c p) d -> p c d", p=128))
        nc.gpsimd.dma_start(Wl_sb[:], Wl_ext.rearrange("(c p) d -> p c d", p=128))

        be_sb = const.tile([128, 2], F32)
        nc.sync.dma_start(be_sb[:], be_ext.rearrange("(c p) -> p c", p=128))

        # broadcast b_mean/b_logvar across partitions via one-time K=1
        # matmuls: [128,CHUNK] = ones[1,128].T @ bias_rep[1,CHUNK]
        ones_sb = const.tile([1, 128], F32)
        nc.vector.memset(ones_sb[:], 1.0)
        bm_rep = const.tile([1, CHUNK], F32)
        bl_rep = const.tile([1, CHUNK], F32)
        for s in range(CHUNK_S):
            nc.sync.dma_start(
                bm_rep[0:1, s * D_OUT : (s + 1) * D_OUT],
                bm_ext.rearrange("(o d) -> o d", o=1),
            )
            nc.sync.dma_start(
                bl_rep[0:1, s * D_OUT : (s + 1) * D_OUT],
                bl_ext.rearrange("(o d) -> o d", o=1),
            )
        bm_bc = const.tile([128, CHUNK], F32)
        bl_bc = const.tile([128, CHUNK], F32)
        binit_ps = psC.tile([128, CHUNK], F32, tag="mean")
        nc.tensor.matmul(binit_ps[:], ones_sb[:], bm_rep[:], start=True, stop=True)
        nc.vector.tensor_copy(bm_bc[:], binit_ps[:])
        binit2_ps = psC.tile([128, CHUNK], F32, tag="lv")
        nc.tensor.matmul(binit2_ps[:], ones_sb[:], bl_rep[:], start=True, stop=True)
        nc.vector.tensor_copy(bl_bc[:], binit2_ps[:])

        # --- pipelined main loop ---
        x_tiles = {}
        eps_tiles = {}

        def fetch(t):
            if t >= n_tiles:
                return
            x_sb = xin.tile([128, S_DMA * D_IN], BF16, tag="x")
            nc.sync.dma_start(x_sb[:], xv[t])
            e_sb = epool.tile([128, S_DMA * D_OUT], BF16, tag="eps")
            nc.sync.dma_start(e_sb[:], ev[t])
            x_tiles[t] = x_sb
            eps_tiles[t] = e_sb

        fetch(0)
        fetch(1)

        # per-chunk live state
        xT_sbs = {}
        out_sbs = {}

        for c in range(n_chunks + 2):
            # ---- stage A: transpose chunk c ----
            if c < n_chunks:
                t, j = divmod(c, CHUNKS_PER_TILE)
                if j == 0:
                    fetch(t + 2)
                x_sb = x_tiles[t]
                xT_ps = psA.tile([128, CHUNK], BF16, tag="xT")
                for q in range(CHUNK_S):
                    s = j * CHUNK_S + q
                    nc.tensor.transpose(
                        xT_ps[:, q * 128 : (q + 1) * 128],
                        x_sb[:, s * D_IN : (s + 1) * D_IN],
                        ident[:],
                    )
                xT_sb = xTp.tile([128, CHUNK], BF16, tag="xTs")
                nc.vector.tensor_copy(xT_sb[:], xT_ps[:])
                xT_sbs[c] = xT_sb
                if j == CHUNKS_PER_TILE - 1:
                    del x_tiles[t]

            # ---- stage B: matmuls + biased outputs for chunk c-1 ----
            d = c - 1
            if 0 <= d < n_chunks:
                xT_sb = xT_sbs.pop(d)
                hT_ps0 = psB.tile([128, CHUNK], F32, tag="hT0")
                hT_ps1 = psB.tile([128, CHUNK], F32, tag="hT1")
                nc.tensor.matmul(
                    hT_ps0[:], We_sb[:, 0:128], xT_sb[:], start=True, stop=True
                )
                nc.tensor.matmul(
                    hT_ps1[:], We_sb[:, 128:256], xT_sb[:], start=True, stop=True
                )
                hT_sb0 = hTp.tile([128, CHUNK], BF16, tag="h0")
                hT_sb1 = hTp.tile([128, CHUNK], BF16, tag="h1")
                nc.scalar.activation(
                    hT_sb0[:],
                    hT_ps0[:],
                    mybir.ActivationFunctionType.Relu,
                    bias=be_sb[:, 0:1],
                )
                nc.scalar.activation(
                    hT_sb1[:],
                    hT_ps1[:],
                    mybir.ActivationFunctionType.Relu,
                    bias=be_sb[:, 1:2],
                )
                mean_ps = psC.tile([128, CHUNK], F32, tag="mean")
                lv_ps = psC.tile([128, CHUNK], F32, tag="lv")
                for s in range(CHUNK_S):
                    sl = slice(s * 128, (s + 1) * 128)
                    so = slice(s * D_OUT, (s + 1) * D_OUT)
                    for k, hT_sbk in ((0, hT_sb0), (1, hT_sb1)):
                        nc.tensor.matmul(
                            mean_ps[:, so],
                            hT_sbk[:, sl],
                            Wm_sb[:, k, :],
                            start=(k == 0),
                            stop=(k == 1),
                        )
                        nc.tensor.matmul(
                            lv_ps[:, so],
                            hT_sbk[:, sl],
                            Wl_sb[:, k, :],
                            start=(k == 0),
                            stop=(k == 1),
                        )

                out_sb = outs.tile([128, 3, CHUNK_S, D_OUT], BF16, tag="o")
                mean_sl = out_sb[:, 1, :, :].rearrange("p s d -> p (s d)")
                lv_sl = out_sb[:, 2, :, :].rearrange("p s d -> p (s d)")
                nc.vector.tensor_add(mean_sl, mean_ps[:], bm_bc[:])
                nc.vector.tensor_add(lv_sl, lv_ps[:], bl_bc[:])
                out_sbs[d] = out_sb

            # ---- stage C: exp / sample / store for chunk c-2 ----
            e = c - 2
            if 0 <= e < n_chunks:
                t_e, j_e = divmod(e, CHUNKS_PER_TILE)
                out_sb = out_sbs.pop(e)
                lv_sl = out_sb[:, 2, :, :].rearrange("p s d -> p (s d)")
                std_sb = stdp.tile([128, CHUNK], BF16, tag="std")
                nc.scalar.activation(
                    std_sb[:], lv_sl, mybir.ActivationFunctionType.Exp, scale=0.5
                )
                e_sb = eps_tiles[t_e]
                se_sb = sep.tile([128, CHUNK], BF16, tag="se")
                nc.gpsimd.tensor_mul(
                    se_sb[:], std_sb[:], e_sb[:, j_e * CHUNK : (j_e + 1) * CHUNK]
                )
                mean_sl = out_sb[:, 1, :, :].rearrange("p s d -> p (s d)")
                z_sl = out_sb[:, 0, :, :].rearrange("p s d -> p (s d)")
                nc.gpsimd.tensor_add(z_sl, mean_sl, se_sb[:])
                nc.sync.dma_start(
                    ov[t_e][:, :, j_e * CHUNK_S : (j_e + 1) * CHUNK_S, :], out_sb[:]
                )
                if j_e == CHUNKS_PER_TILE - 1:
                    del eps_tiles[t_e]

    nc.finalize()
    return nc


_NC_CACHE = None


def _get_nc():
    global _NC_CACHE
    if _NC_CACHE is None:
        _NC_CACHE = build_bass()
    return _NC_CACHE


def _run(inputs, trace=False, **kw):
    nc = _get_nc()
    xs = np.ascontiguousarray(np.asarray(inputs["x"])).astype(NPBF16)
    es = np.ascontiguousarray(np.asarray(inputs["eps"])).astype(NPBF16)
    weights = {
        k: np.ascontiguousarray(np.asarray(inputs[k], dtype=np.float32))
        for k in ("W_emb", "b_emb", "W_mean", "b_mean", "W_logvar", "b_logvar")
    }
    in_maps = []
    for c in range(N_CORES):
        sl = slice(c * ROWS_PER_CORE, (c + 1) * ROWS_PER_CORE)
        in_maps.append({"x": xs[sl], "eps": es[sl], **weights})
    res = run_bass_kernel_spmd(nc, in_maps, list(range(N_CORES)), trace=trace, **kw)
    z = np.concatenate(
        [res.results[c]["out"][0] for c in range(N_CORES)], axis=0
    ).astype(np.float32)
    mean = np.concatenate(
        [res.results[c]["out"][1] for c in range(N_CORES)], axis=0
    ).astype(np.float32)
    lv = np.concatenate(
        [res.results[c]["out"][2] for c in range(N_CORES)], axis=0
    ).astype(np.float32)
    return (z, mean, lv), res


def kernel(**inputs):
    out, _ = _run(inputs, trace=False)
    return out


if __name__ == "__main__":
    rng = np.random.default_rng(0)
    demo = {
        "x": rng.standard_normal((B, D_IN), dtype=np.float32),
        "eps": rng.standard_normal((B, D_OUT), dtype=np.float32),
        "W_emb": rng.standard_normal((D_IN, D_H), dtype=np.float32) * 0.088,
        "b_emb": rng.standard_normal((D_H,), dtype=np.float32) * 0.05,
        "W_mean": rng.standard_normal((D_H, D_OUT), dtype=np.float32) * 0.06,
        "b_mean": rng.standard_normal((D_OUT,), dtype=np.float32) * 0.03,
        "W_logvar": rng.standard_normal((D_H, D_OUT), dtype=np.float32) * 0.06,
        "b_logvar": rng.standard_normal((D_OUT,), dtype=np.float32) * 0.03,
    }
    z, m, l = kernel(**demo)
    print("shapes", z.shape, m.shape, l.shape)


# revision 12
# speedup vs baseline: 2.7123x; 1.4216x over previous
"""GaussianMLP sampling kernel for 8 trn2 NeuronCores (pure data parallel).

reference:
    h      = relu(x @ W_emb + b_emb)        x:[B,128] W_emb:[128,256]
    mean   = h @ W_mean + b_mean            W_mean:[256,128]
    logvar = h @ W_logvar + b_logvar        W_logvar:[256,128]
    z      = mean + exp(0.5*logvar) * eps
    returns (z, mean, logvar)

Sharding: x/eps split along batch across 8 cores; weights replicated.

v3 design (memory-regime):
  - All bulk I/O in bf16 (host converts): halves HBM traffic. Outputs are
    packed into one [3, R, 128] DRAM tensor, written with ONE DMA per
    2048-row tile (4 KiB contiguous runs per partition).
  - DRAM views "(t p s) d -> t p (s d)" keep per-partition runs >= 4 KiB.
  - PE per 512-row chunk: 4 bf16 transposes (512 cyc) + L1 (1024 cyc) +
    L2 (2048 cyc, 8 matmuls of 256 cols into a combined [mean|logvar]
    PSUM tile). No bias matmuls: L1 bias rides the ACT relu; L2 biases
    are added by DVE/Pool from precomputed broadcast tiles.
  - 5-stage software pipeline so every engine runs dependency-free:
      A: transpose(c) [PE] + PSUM->SBUF copy [DVE]
      B: L1(c-1) [PE] + relu0/1(c-1) [ACT]
      C: L2(c-2) [PE]
      D: +b_mean(c-3) [DVE], +b_logvar(c-3) [Pool]
      E: exp(c-4) [ACT], se=std*eps(c-4) [DVE], z=mean+se(c-4) [DVE],
         output DMA (per tile) [Pool SWDGE queue]
"""

import sys

sys.path.insert(0, "/opt/trn_rl_repo")

import numpy as np
import ml_dtypes

from contextlib import ExitStack

from concourse import bacc, bass, masks, mybir, tile
from concourse.alu_op_type import AluOpType
from concourse.bass_utils import run_bass_kernel_spmd

N_CORES = 8
B = 524288
D_IN = 128
D_H = 256
D_OUT = 128
ROWS_PER_CORE = B // N_CORES  # 65536

S_DMA = 16  # rows-per-partition per input DMA tile (2048 rows)
CHUNK_S = 4  # 512-row compute chunk = 4 x 128-row subtiles
CHUNK = CHUNK_S * 128
CHUNKS_PER_TILE = S_DMA // CHUNK_S  # 4
TILE_ROWS = 128 * S_DMA  # 2048

F32 = mybir.dt.float32
BF16 = mybir.dt.bfloat16
NPBF16 = ml_dtypes.bfloat16


def build_bass(rows_per_core=ROWS_PER_CORE):
    nc = bacc.Bacc("TRN2", target_bir_lowering=False, debug=False)
    n_tiles = rows_per_core // TILE_ROWS
    n_chunks = rows_per_core // CHUNK

    x_ext = nc.declare_dram_parameter("x", [rows_per_core, D_IN], BF16, isOutput=False)
    eps_ext = nc.declare_dram_parameter(
        "eps", [rows_per_core, D_OUT], BF16, isOutput=False
    )
    We_ext = nc.declare_dram_parameter("W_emb", [D_IN, D_H], F32, isOutput=False)
    be_ext = nc.declare_dram_parameter("b_emb", [D_H], F32, isOutput=False)
    Wm_ext = nc.declare_dram_parameter("W_mean", [D_H, D_OUT], F32, isOutput=False)
    bm_ext = nc.declare_dram_parameter("b_mean", [D_OUT], F32, isOutput=False)
    Wl_ext = nc.declare_dram_parameter("W_logvar", [D_H, D_OUT], F32, isOutput=False)
    bl_ext = nc.declare_dram_parameter("b_logvar", [D_OUT], F32, isOutput=False)
    out_ext = nc.declare_dram_parameter(
        "out", [3, rows_per_core, D_OUT], BF16, isOutput=True
    )

    # row = t*TILE_ROWS + p*S_DMA + s ; per-partition contiguous run = s*d
    xv = x_ext.rearrange("(t p s) d -> t p (s d)", p=128, s=S_DMA)
    ev = eps_ext.rearrange("(t p s) d -> t p (s d)", p=128, s=S_DMA)
    ov = out_ext.rearrange("c (t p s) d -> t p c s d", p=128, s=S_DMA)

    with tile.TileContext(nc) as tc, ExitStack() as ctx:
        const = ctx.enter_context(tc.tile_pool(name="const", bufs=1))
        xin = ctx.enter_context(tc.tile_pool(name="xin", bufs=3))
        epool = ctx.enter_context(tc.tile_pool(name="eps", bufs=4))
        xTp = ctx.enter_context(tc.tile_pool(name="xT", bufs=3))
        hTp = ctx.enter_context(tc.tile_pool(name="hTs", bufs=3))
        stdp = ctx.enter_context(tc.tile_pool(name="std", bufs=2))
        sep = ctx.enter_context(tc.tile_pool(name="se", bufs=2))
        outs = ctx.enter_context(tc.tile_pool(name="outs", bufs=2))
        psA = ctx.enter_context(tc.tile_pool(name="psA", bufs=2, space="PSUM"))
        psB = ctx.enter_context(tc.tile_pool(name="psB", bufs=1, space="PSUM"))
        psC = ctx.enter_context(tc.tile_pool(name="psC", bufs=2, space="PSUM"))

        # --- constants / weights (loaded once) ---
        ident = const.tile([128, 128], BF16)
        masks.make_identity(nc, ident[:])

        We_sb = const.tile([128, D_H], BF16)
        nc.gpsimd.dma_start(We_sb[:], We_ext[:])
        # combined [W_mean | W_logvar]: [k-chunk partition, k, 2*D_OUT]
        Wml_sb = const.tile([128, 2, 2 * D_OUT], BF16)
        nc.gpsimd.dma_start(
            Wml_sb[:, :, 0:D_OUT], Wm_ext.rearrange("(c p) d -> p c d", p=128)
        )
        nc.gpsimd.dma_start(
            Wml_sb[:, :, D_OUT : 2 * D_OUT],
            Wl_ext.rearrange("(c p) d -> p c d", p=128),
        )

        be_sb = const.tile([128, 2], F32)
        nc.sync.dma_start(be_sb[:], be_ext.rearrange("(c p) -> p c", p=128))

        # broadcast b_mean/b_logvar across partitions via one-time K=1
        # matmuls: [128,CHUNK] = ones[1,128].T @ bias_rep[1,CHUNK]
        ones_sb = const.tile([1, 128], F32)
        nc.vector.memset(ones_sb[:], 1.0)
        bm_rep = const.tile([1, CHUNK], F32)
        bl_rep = const.tile([1, CHUNK], F32)
        for s in range(CHUNK_S):
            nc.sync.dma_start(
                bm_rep[0:1, s * D_OUT : (s + 1) * D_OUT],
                bm_ext.rearrange("(o d) -> o d", o=1),
            )
            nc.sync.dma_start(
                bl_rep[0:1, s * D_OUT : (s + 1) * D_OUT],
                bl_ext.rearrange("(o d) -> o d", o=1),
            )
        bm_bc = const.tile([128, CHUNK_S, D_OUT], F32)
        bl_bc = const.tile([128, CHUNK_S, D_OUT], F32)
        binit_ps = psC.tile([128, CHUNK_S, 2 * D_OUT], F32, tag="ml")
        nc.tensor.matmul(
            binit_ps[:].rearrange("p s d -> p (s d)")[:, 0:CHUNK],
            ones_sb[:],
            bm_rep[:],
            start=True,
            stop=True,
            skip_group_check=True,
        )
        nc.vector.tensor_copy(
            bm_bc[:].rearrange("p s d -> p (s d)"),
            binit_ps[:].rearrange("p s d -> p (s d)")[:, 0:CHUNK],
        )
        binit2_ps = psC.tile([128, CHUNK_S, 2 * D_OUT], F32, tag="ml")
        nc.tensor.matmul(
            binit2_ps[:].rearrange("p s d -> p (s d)")[:, 0:CHUNK],
            ones_sb[:],
            bl_rep[:],
            start=True,
            stop=True,
            skip_group_check=True,
        )
        nc.vector.tensor_copy(
            bl_bc[:].rearrange("p s d -> p (s d)"),
            binit2_ps[:].rearrange("p s d -> p (s d)")[:, 0:CHUNK],
        )

        # --- pipelined main loop ---
        x_tiles = {}
        eps_tiles = {}
        xT_sbs = {}
        hT_sbs = {}
        ml_pss = {}
        out_sbs = {}
        std_sbs = {}

        def fetch(t):
            if t >= n_tiles:
                return
            x_sb = xin.tile([128, S_DMA * D_IN], BF16, tag="x")
            nc.sync.dma_start(x_sb[:], xv[t])
            e_sb = epool.tile([128, S_DMA * D_OUT], BF16, tag="eps")
            nc.sync.dma_start(e_sb[:], ev[t])
            x_tiles[t] = x_sb
            eps_tiles[t] = e_sb

        fetch(0)
        fetch(1)

        for c in range(n_chunks + 5):
            # ---- stage A: transpose chunk c ----
            if c < n_chunks:
                t, j = divmod(c, CHUNKS_PER_TILE)
                if j == 0:
                    fetch(t + 2)
                x_sb = x_tiles[t]
                xT_ps = psA.tile([128, CHUNK], BF16, tag="xT")
                for q in range(CHUNK_S):
                    s = j * CHUNK_S + q
                    nc.tensor.transpose(
                        xT_ps[:, q * 128 : (q + 1) * 128],
                        x_sb[:, s * D_IN : (s + 1) * D_IN],
                        ident[:],
                    )
                xT_sb = xTp.tile([128, CHUNK], BF16, tag="xTs")
                nc.vector.tensor_copy(xT_sb[:], xT_ps[:])
                xT_sbs[c] = xT_sb
                if j == CHUNKS_PER_TILE - 1:
                    del x_tiles[t]

            # ---- stage B: L1 + relu for chunk c-1 ----
            d = c - 1
            if 0 <= d < n_chunks:
                xT_sb = xT_sbs.pop(d)
                hT_ps0 = psB.tile([128, CHUNK], F32, tag="hT0")
                hT_ps1 = psB.tile([128, CHUNK], F32, tag="hT1")
                nc.tensor.matmul(
                    hT_ps0[:], We_sb[:, 0:128], xT_sb[:], start=True, stop=True
                )
                nc.tensor.matmul(
                    hT_ps1[:], We_sb[:, 128:256], xT_sb[:], start=True, stop=True
                )
                hT_sb0 = hTp.tile([128, CHUNK], BF16, tag="h0")
                hT_sb1 = hTp.tile([128, CHUNK], BF16, tag="h1")
                nc.scalar.activation(
                    hT_sb0[:],
                    hT_ps0[:],
                    mybir.ActivationFunctionType.Relu,
                    bias=be_sb[:, 0:1],
                )
                nc.scalar.activation(
                    hT_sb1[:],
                    hT_ps1[:],
                    mybir.ActivationFunctionType.Relu,
                    bias=be_sb[:, 1:2],
                )
                hT_sbs[d] = (hT_sb0, hT_sb1)

            # ---- stage C: L2 for chunk c-2 ----
            e = c - 2
            if 0 <= e < n_chunks:
                hT_sb0, hT_sb1 = hT_sbs.pop(e)
                ml_ps = psC.tile([128, CHUNK_S, 2 * D_OUT], F32, tag="ml")
                for s in range(CHUNK_S):
                    sl = slice(s * 128, (s + 1) * 128)
                    for k, hT_sbk in ((0, hT_sb0), (1, hT_sb1)):
                        nc.tensor.matmul(
                            ml_ps[:, s, :],
                            hT_sbk[:, sl],
                            Wml_sb[:, k, :],
                            start=(k == 0),
                            stop=(k == 1),
                        )
                ml_pss[e] = ml_ps

            # ---- stage D: bias adds for chunk c-3 ----
            f = c - 3
            if 0 <= f < n_chunks:
                t_f, j_f = divmod(f, CHUNKS_PER_TILE)
                if j_f == 0:
                    out_sbs[t_f] = outs.tile(
                        [128, 3, S_DMA, D_OUT], BF16, tag="o", name="out_sb"
                    )
                out_sb = out_sbs[t_f]
                ml_ps = ml_pss.pop(f)
                ssl = slice(j_f * CHUNK_S, (j_f + 1) * CHUNK_S)
                nc.vector.tensor_add(
                    out_sb[:, 1, ssl, :], ml_ps[:, :, 0:D_OUT], bm_bc[:]
                )
                nc.vector.tensor_add(
                    out_sb[:, 2, ssl, :], ml_ps[:, :, D_OUT : 2 * D_OUT], bl_bc[:]
                )

            # ---- stage E: exp / sample / store for chunk c-4 ----
            g = c - 4
            if 0 <= g < n_chunks:
                t_g, j_g = divmod(g, CHUNKS_PER_TILE)
                out_sb = out_sbs[t_g]
                ssl = slice(j_g * CHUNK_S, (j_g + 1) * CHUNK_S)
                std_sb = stdp.tile([128, CHUNK_S, D_OUT], BF16, tag="std")
                nc.scalar.activation(
                    std_sb[:],
                    out_sb[:, 2, ssl, :],
                    mybir.ActivationFunctionType.Exp,
                    scale=0.5,
                )
                e_sb = eps_tiles[t_g]
                se_sb = sep.tile([128, CHUNK_S, D_OUT], BF16, tag="se")
                nc.vector.tensor_mul(
                    se_sb[:],
                    std_sb[:],
                    e_sb[:, j_g * CHUNK : (j_g + 1) * CHUNK].rearrange(
                        "p (s d) -> p s d", s=CHUNK_S
                    ),
                )
                nc.gpsimd.tensor_add(
                    out_sb[:, 0, ssl, :], out_sb[:, 1, ssl, :], se_sb[:]
                )
                if j_g == CHUNKS_PER_TILE - 1:
                    nc.gpsimd.dma_start(ov[t_g], out_sb[:])
                    del out_sbs[t_g]
                    del eps_tiles[t_g]

    nc.finalize()
    return nc


_NC_CACHE = None


def _get_nc():
    global _NC_CACHE
    if _NC_CACHE is None:
        _NC_CACHE = build_bass()
    return _NC_CACHE


def _run(inputs, trace=False, **kw):
    nc = _get_nc()
    xs = np.ascontiguousarray(np.asarray(inputs["x"])).astype(NPBF16)
    es = np.ascontiguousarray(np.asarray(inputs["eps"])).astype(NPBF16)
    weights = {
        k: np.ascontiguousarray(np.asarray(inputs[k], dtype=np.float32))
        for k in ("W_emb", "b_emb", "W_mean", "b_mean", "W_logvar", "b_logvar")
    }
    in_maps = []
    for c in range(N_CORES):
        sl = slice(c * ROWS_PER_CORE, (c + 1) * ROWS_PER_CORE)
        in_maps.append({"x": xs[sl], "eps": es[sl], **weights})
    res = run_bass_kernel_spmd(nc, in_maps, list(range(N_CORES)), trace=trace, **kw)
    z = np.concatenate(
        [res.results[c]["out"][0] for c in range(N_CORES)], axis=0
    ).astype(np.float32)
    mean = np.concatenate(
        [res.results[c]["out"][1] for c in range(N_CORES)], axis=0
    ).astype(np.float32)
    lv = np.concatenate(
        [res.results[c]["out"][2] for c in range(N_CORES)], axis=0
    ).astype(np.float32)
    return (z, mean, lv), res


def kernel(**inputs):
    out, _ = _run(inputs, trace=False)
    return out


if __name__ == "__main__":
    rng = np.random.default_rng(0)
    demo = {
        "x": rng.standard_normal((B, D_IN), dtype=np.float32),
        "eps": rng.standard_normal((B, D_OUT), dtype=np.float32),
        "W_emb": rng.standard_normal((D_IN, D_H), dtype=np.float32) * 0.088,
        "b_emb": rng.standard_normal((D_H,), dtype=np.float32) * 0.05,
        "W_mean": rng.standard_normal((D_H, D_OUT), dtype=np.float32) * 0.06,
        "b_mean": rng.standard_normal((D_OUT,), dtype=np.float32) * 0.03,
        "W_logvar": rng.standard_normal((D_H, D_OUT), dtype=np.float32) * 0.06,
        "b_logvar": rng.standard_normal((D_OUT,), dtype=np.float32) * 0.03,
    }
    z, m, l = kernel(**demo)
    print("shapes", z.shape, m.shape, l.shape)


# revision 14
# speedup vs baseline: 2.7388x; 1.0098x over previous
"""GaussianMLP sampling kernel for 8 trn2 NeuronCores (pure data parallel).

reference:
    h      = relu(x @ W_emb + b_emb)        x:[B,128] W_emb:[128,256]
    mean   = h @ W_mean + b_mean            W_mean:[256,128]
    logvar = h @ W_logvar + b_logvar        W_logvar:[256,128]
    z      = mean + exp(0.5*logvar) * eps
    returns (z, mean, logvar)

Sharding: x/eps split along batch across 8 cores; weights replicated.

v3 design (memory-regime):
  - All bulk I/O in bf16 (host converts): halves HBM traffic. Outputs are
    packed into one [3, R, 128] DRAM tensor, written with ONE DMA per
    2048-row tile (4 KiB contiguous runs per partition).
  - DRAM views "(t p s) d -> t p (s d)" keep per-partition runs >= 4 KiB.
  - PE per 512-row chunk: 4 bf16 transposes (512 cyc) + L1 (1024 cyc) +
    L2 (2048 cyc, 8 matmuls of 256 cols into a combined [mean|logvar]
    PSUM tile). No bias matmuls: L1 bias rides the ACT relu; L2 biases
    are added by DVE/Pool from precomputed broadcast tiles.
  - 5-stage software pipeline so every engine runs dependency-free:
      A: transpose(c) [PE] + PSUM->SBUF copy [DVE]
      B: L1(c-1) [PE] + relu0/1(c-1) [ACT]
      C: L2(c-2) [PE]
      D: +b_mean(c-3) [DVE], +b_logvar(c-3) [Pool]
      E: exp(c-4) [ACT], se=std*eps(c-4) [DVE], z=mean+se(c-4) [DVE],
         output DMA (per tile) [Pool SWDGE queue]
"""

import sys

sys.path.insert(0, "/opt/trn_rl_repo")

import numpy as np
import ml_dtypes

from contextlib import ExitStack

from concourse import bacc, bass, masks, mybir, tile
from concourse.alu_op_type import AluOpType
from concourse.bass_utils import run_bass_kernel_spmd

N_CORES = 8
B = 524288
D_IN = 128
D_H = 256
D_OUT = 128
ROWS_PER_CORE = B // N_CORES  # 65536

S_DMA = 16  # rows-per-partition per input DMA tile (2048 rows)
CHUNK_S = 4  # 512-row compute chunk = 4 x 128-row subtiles
CHUNK = CHUNK_S * 128
CHUNKS_PER_TILE = S_DMA // CHUNK_S  # 4
TILE_ROWS = 128 * S_DMA  # 2048

F32 = mybir.dt.float32
BF16 = mybir.dt.bfloat16
NPBF16 = ml_dtypes.bfloat16


def build_bass(rows_per_core=ROWS_PER_CORE):
    nc = bacc.Bacc("TRN2", target_bir_lowering=False, debug=False)
    n_tiles = rows_per_core // TILE_ROWS
    n_chunks = rows_per_core // CHUNK

    x_ext = nc.declare_dram_parameter("x", [rows_per_core, D_IN], BF16, isOutput=False)
    eps_ext = nc.declare_dram_parameter(
        "eps", [rows_per_core, D_OUT], BF16, isOutput=False
    )
    We_ext = nc.declare_dram_parameter("W_emb", [D_IN, D_H], F32, isOutput=False)
    be_ext = nc.declare_dram_parameter("b_emb", [D_H], F32, isOutput=False)
    Wm_ext = nc.declare_dram_parameter("W_mean", [D_H, D_OUT], F32, isOutput=False)
    bm_ext = nc.declare_dram_parameter("b_mean", [D_OUT], F32, isOutput=False)
    Wl_ext = nc.declare_dram_parameter("W_logvar", [D_H, D_OUT], F32, isOutput=False)
    bl_ext = nc.declare_dram_parameter("b_logvar", [D_OUT], F32, isOutput=False)
    out_ext = nc.declare_dram_parameter(
        "out", [3, rows_per_core, D_OUT], BF16, isOutput=True
    )

    # row = t*TILE_ROWS + p*S_DMA + s ; per-partition contiguous run = s*d
    xv = x_ext.rearrange("(t p s) d -> t p (s d)", p=128, s=S_DMA)
    ev = eps_ext.rearrange("(t p s) d -> t p (s d)", p=128, s=S_DMA)
    ov = out_ext.rearrange("c (t p s) d -> t p c s d", p=128, s=S_DMA)

    with tile.TileContext(nc) as tc, ExitStack() as ctx:
        const = ctx.enter_context(tc.tile_pool(name="const", bufs=1))
        xin = ctx.enter_context(tc.tile_pool(name="xin", bufs=3))
        epool = ctx.enter_context(tc.tile_pool(name="eps", bufs=4))
        xTp = ctx.enter_context(tc.tile_pool(name="xT", bufs=3))
        hTp = ctx.enter_context(tc.tile_pool(name="hTs", bufs=3))
        stdp = ctx.enter_context(tc.tile_pool(name="std", bufs=2))
        sep = ctx.enter_context(tc.tile_pool(name="se", bufs=2))
        outs = ctx.enter_context(tc.tile_pool(name="outs", bufs=2))
        psA = ctx.enter_context(tc.tile_pool(name="psA", bufs=2, space="PSUM"))
        psB = ctx.enter_context(tc.tile_pool(name="psB", bufs=1, space="PSUM"))
        psC = ctx.enter_context(tc.tile_pool(name="psC", bufs=2, space="PSUM"))

        # --- constants / weights (loaded once) ---
        ident = const.tile([128, 128], BF16)
        masks.make_identity(nc, ident[:])

        We_sb = const.tile([128, D_H], BF16)
        nc.gpsimd.dma_start(We_sb[:], We_ext[:])
        # combined [W_mean | W_logvar]: [k-chunk partition, k, 2*D_OUT]
        Wml_sb = const.tile([128, 2, 2 * D_OUT], BF16)
        nc.gpsimd.dma_start(
            Wml_sb[:, :, 0:D_OUT], Wm_ext.rearrange("(c p) d -> p c d", p=128)
        )
        nc.gpsimd.dma_start(
            Wml_sb[:, :, D_OUT : 2 * D_OUT],
            Wl_ext.rearrange("(c p) d -> p c d", p=128),
        )

        be_sb = const.tile([128, 2], F32)
        nc.sync.dma_start(be_sb[:], be_ext.rearrange("(c p) -> p c", p=128))

        # broadcast b_mean/b_logvar across partitions via one-time K=1
        # matmuls: [128,CHUNK] = ones[1,128].T @ bias_rep[1,CHUNK]
        ones_sb = const.tile([1, 128], F32)
        nc.vector.memset(ones_sb[:], 1.0)
        bm_rep = const.tile([1, CHUNK], F32)
        bl_rep = const.tile([1, CHUNK], F32)
        for s in range(CHUNK_S):
            nc.sync.dma_start(
                bm_rep[0:1, s * D_OUT : (s + 1) * D_OUT],
                bm_ext.rearrange("(o d) -> o d", o=1),
            )
            nc.sync.dma_start(
                bl_rep[0:1, s * D_OUT : (s + 1) * D_OUT],
                bl_ext.rearrange("(o d) -> o d", o=1),
            )
        bm_bc = const.tile([128, CHUNK_S, D_OUT], F32)
        bl_bc = const.tile([128, CHUNK_S, D_OUT], F32)
        binit_ps = psC.tile([128, CHUNK_S, 2 * D_OUT], F32, tag="ml")
        nc.tensor.matmul(
            binit_ps[:].rearrange("p s d -> p (s d)")[:, 0:CHUNK],
            ones_sb[:],
            bm_rep[:],
            start=True,
            stop=True,
            skip_group_check=True,
        )
        nc.vector.tensor_copy(
            bm_bc[:].rearrange("p s d -> p (s d)"),
            binit_ps[:].rearrange("p s d -> p (s d)")[:, 0:CHUNK],
        )
        binit2_ps = psC.tile([128, CHUNK_S, 2 * D_OUT], F32, tag="ml")
        nc.tensor.matmul(
            binit2_ps[:].rearrange("p s d -> p (s d)")[:, 0:CHUNK],
            ones_sb[:],
            bl_rep[:],
            start=True,
            stop=True,
            skip_group_check=True,
        )
        nc.vector.tensor_copy(
            bl_bc[:].rearrange("p s d -> p (s d)"),
            binit2_ps[:].rearrange("p s d -> p (s d)")[:, 0:CHUNK],
        )

        # --- pipelined main loop ---
        x_tiles = {}
        eps_tiles = {}
        xT_sbs = {}
        hT_sbs = {}
        ml_pss = {}
        out_sbs = {}
        std_sbs = {}

        def fetch(t):
            if t >= n_tiles:
                return
            x_sb = xin.tile([128, S_DMA * D_IN], BF16, tag="x")
            nc.sync.dma_start(x_sb[:], xv[t])
            e_sb = epool.tile([128, S_DMA * D_OUT], BF16, tag="eps")
            nc.sync.dma_start(e_sb[:], ev[t])
            x_tiles[t] = x_sb
            eps_tiles[t] = e_sb

        fetch(0)
        fetch(1)

        for c in range(n_chunks + 5):
            # ---- stage E part 1: exp for chunk c-4 (inputs ready since
            # last iteration -> keeps ACT dense from the iteration start) ----
            g = c - 4
            if 0 <= g < n_chunks:
                t_g, j_g = divmod(g, CHUNKS_PER_TILE)
                out_sb_g = out_sbs[t_g]
                ssl_g = slice(j_g * CHUNK_S, (j_g + 1) * CHUNK_S)
                std_sb = stdp.tile([128, CHUNK_S, D_OUT], BF16, tag="std")
                nc.scalar.activation(
                    std_sb[:],
                    out_sb_g[:, 2, ssl_g, :],
                    mybir.ActivationFunctionType.Exp,
                    scale=0.5,
                )
                std_sbs[g] = std_sb

            # ---- stage D: bias adds for chunk c-3 (L2 done last iter,
            # so these are ready first on DVE) ----
            f = c - 3
            if 0 <= f < n_chunks:
                t_f, j_f = divmod(f, CHUNKS_PER_TILE)
                if j_f == 0:
                    out_sbs[t_f] = outs.tile(
                        [128, 3, S_DMA, D_OUT], BF16, tag="o", name="out_sb"
                    )
                out_sb = out_sbs[t_f]
                ml_ps = ml_pss.pop(f)
                ssl = slice(j_f * CHUNK_S, (j_f + 1) * CHUNK_S)
                nc.vector.tensor_add(
                    out_sb[:, 1, ssl, :], ml_ps[:, :, 0:D_OUT], bm_bc[:]
                )
                nc.vector.tensor_add(
                    out_sb[:, 2, ssl, :], ml_ps[:, :, D_OUT : 2 * D_OUT], bl_bc[:]
                )

            # ---- stage A: transpose chunk c ----
            if c < n_chunks:
                t, j = divmod(c, CHUNKS_PER_TILE)
                if j == 0:
                    fetch(t + 2)
                x_sb = x_tiles[t]
                xT_ps = psA.tile([128, CHUNK], BF16, tag="xT")
                for q in range(CHUNK_S):
                    s = j * CHUNK_S + q
                    nc.tensor.transpose(
                        xT_ps[:, q * 128 : (q + 1) * 128],
                        x_sb[:, s * D_IN : (s + 1) * D_IN],
                        ident[:],
                    )
                xT_sb = xTp.tile([128, CHUNK], BF16, tag="xTs")
                nc.vector.tensor_copy(xT_sb[:], xT_ps[:])
                xT_sbs[c] = xT_sb
                if j == CHUNKS_PER_TILE - 1:
                    del x_tiles[t]

            # ---- stage B: L1 + relu for chunk c-1 ----
            d = c - 1
            if 0 <= d < n_chunks:
                xT_sb = xT_sbs.pop(d)
                hT_ps0 = psB.tile([128, CHUNK], F32, tag="hT0")
                hT_ps1 = psB.tile([128, CHUNK], F32, tag="hT1")
                nc.tensor.matmul(
                    hT_ps0[:], We_sb[:, 0:128], xT_sb[:], start=True, stop=True
                )
                nc.tensor.matmul(
                    hT_ps1[:], We_sb[:, 128:256], xT_sb[:], start=True, stop=True
                )
                hT_sb0 = hTp.tile([128, CHUNK], BF16, tag="h0")
                hT_sb1 = hTp.tile([128, CHUNK], BF16, tag="h1")
                nc.scalar.activation(
                    hT_sb0[:],
                    hT_ps0[:],
                    mybir.ActivationFunctionType.Relu,
                    bias=be_sb[:, 0:1],
                )
                nc.scalar.activation(
                    hT_sb1[:],
                    hT_ps1[:],
                    mybir.ActivationFunctionType.Relu,
                    bias=be_sb[:, 1:2],
                )
                hT_sbs[d] = (hT_sb0, hT_sb1)

            # ---- stage C: L2 for chunk c-2 ----
            e = c - 2
            if 0 <= e < n_chunks:
                hT_sb0, hT_sb1 = hT_sbs.pop(e)
                ml_ps = psC.tile([128, CHUNK_S, 2 * D_OUT], F32, tag="ml")
                for s in range(CHUNK_S):
                    sl = slice(s * 128, (s + 1) * 128)
                    for k, hT_sbk in ((0, hT_sb0), (1, hT_sb1)):
                        nc.tensor.matmul(
                            ml_ps[:, s, :],
                            hT_sbk[:, sl],
                            Wml_sb[:, k, :],
                            start=(k == 0),
                            stop=(k == 1),
                        )
                ml_pss[e] = ml_ps

            # ---- stage E part 2: sample / store for chunk c-4 ----
            if 0 <= g < n_chunks:
                t_g, j_g = divmod(g, CHUNKS_PER_TILE)
                out_sb = out_sbs[t_g]
                ssl = slice(j_g * CHUNK_S, (j_g + 1) * CHUNK_S)
                std_sb = std_sbs.pop(g)
                e_sb = eps_tiles[t_g]
                se_sb = sep.tile([128, CHUNK_S, D_OUT], BF16, tag="se")
                nc.vector.tensor_mul(
                    se_sb[:],
                    std_sb[:],
                    e_sb[:, j_g * CHUNK : (j_g + 1) * CHUNK].rearrange(
                        "p (s d) -> p s d", s=CHUNK_S
                    ),
                )
                nc.gpsimd.tensor_add(
                    out_sb[:, 0, ssl, :], out_sb[:, 1, ssl, :], se_sb[:]
                )
                if j_g == CHUNKS_PER_TILE - 1:
                    nc.gpsimd.dma_start(ov[t_g], out_sb[:])
                    del out_sbs[t_g]
                    del eps_tiles[t_g]

    nc.finalize()
    return nc


_NC_CACHE = None


def _get_nc():
    global _NC_CACHE
    if _NC_CACHE is None:
        _NC_CACHE = build_bass()
    return _NC_CACHE


def _run(inputs, trace=False, **kw):
    nc = _get_nc()
    xs = np.ascontiguousarray(np.asarray(inputs["x"])).astype(NPBF16)
    es = np.ascontiguousarray(np.asarray(inputs["eps"])).astype(NPBF16)
    weights = {
        k: np.ascontiguousarray(np.asarray(inputs[k], dtype=np.float32))
        for k in ("W_emb", "b_emb", "W_mean", "b_mean", "W_logvar", "b_logvar")
    }
    in_maps = []
    for c in range(N_CORES):
        sl = slice(c * ROWS_PER_CORE, (c + 1) * ROWS_PER_CORE)
        in_maps.append({"x": xs[sl], "eps": es[sl], **weights})
    res = run_bass_kernel_spmd(nc, in_maps, list(range(N_CORES)), trace=trace, **kw)
    z = np.concatenate(
        [res.results[c]["out"][0] for c in range(N_CORES)], axis=0
    ).astype(np.float32)
    mean = np.concatenate(
        [res.results[c]["out"][1] for c in range(N_CORES)], axis=0
    ).astype(np.float32)
    lv = np.concatenate(
        [res.results[c]["out"][2] for c in range(N_CORES)], axis=0
    ).astype(np.float32)
    return (z, mean, lv), res


def kernel(**inputs):
    out, _ = _run(inputs, trace=False)
    return out


if __name__ == "__main__":
    rng = np.random.default_rng(0)
    demo = {
        "x": rng.standard_normal((B, D_IN), dtype=np.float32),
        "eps": rng.standard_normal((B, D_OUT), dtype=np.float32),
        "W_emb": rng.standard_normal((D_IN, D_H), dtype=np.float32) * 0.088,
        "b_emb": rng.standard_normal((D_H,), dtype=np.float32) * 0.05,
        "W_mean": rng.standard_normal((D_H, D_OUT), dtype=np.float32) * 0.06,
        "b_mean": rng.standard_normal((D_OUT,), dtype=np.float32) * 0.03,
        "W_logvar": rng.standard_normal((D_H, D_OUT), dtype=np.float32) * 0.06,
        "b_logvar": rng.standard_normal((D_OUT,), dtype=np.float32) * 0.03,
    }
    z, m, l = kernel(**demo)
    print("shapes", z.shape, m.shape, l.shape)


# revision 16
# speedup vs baseline: 2.9662x; 1.0830x over previous
"""GaussianMLP sampling kernel for 8 trn2 NeuronCores (pure data parallel).

reference:
    h      = relu(x @ W_emb + b_emb)        x:[B,128] W_emb:[128,256]
    mean   = h @ W_mean + b_mean            W_mean:[256,128]
    logvar = h @ W_logvar + b_logvar        W_logvar:[256,128]
    z      = mean + exp(0.5*logvar) * eps
    returns (z, mean, logvar)

Sharding: x/eps split along batch across 8 cores; weights replicated.

v3 design (memory-regime):
  - All bulk I/O in bf16 (host converts): halves HBM traffic. Outputs are
    packed into one [3, R, 128] DRAM tensor, written with ONE DMA per
    2048-row tile (4 KiB contiguous runs per partition).
  - DRAM views "(t p s) d -> t p (s d)" keep per-partition runs >= 4 KiB.
  - PE per 512-row chunk: 4 bf16 transposes (512 cyc) + L1 (1024 cyc) +
    L2 (2048 cyc, 8 matmuls of 256 cols into a combined [mean|logvar]
    PSUM tile). No bias matmuls: L1 bias rides the ACT relu; L2 biases
    are added by DVE/Pool from precomputed broadcast tiles.
  - 5-stage software pipeline so every engine runs dependency-free:
      A: transpose(c) [PE] + PSUM->SBUF copy [DVE]
      B: L1(c-1) [PE] + relu0/1(c-1) [ACT]
      C: L2(c-2) [PE]
      D: +b_mean(c-3) [DVE], +b_logvar(c-3) [Pool]
      E: exp(c-4) [ACT], se=std*eps(c-4) [DVE], z=mean+se(c-4) [DVE],
         output DMA (per tile) [Pool SWDGE queue]
"""

import sys

sys.path.insert(0, "/opt/trn_rl_repo")

import numpy as np
import ml_dtypes

from contextlib import ExitStack

from concourse import bacc, bass, masks, mybir, tile
from concourse.alu_op_type import AluOpType
from concourse.bass_utils import run_bass_kernel_spmd

N_CORES = 8
B = 524288
D_IN = 128
D_H = 256
D_OUT = 128
ROWS_PER_CORE = B // N_CORES  # 65536

S_DMA = 16  # rows-per-partition per input DMA tile (2048 rows)
CHUNK_S = 4  # 512-row compute chunk = 4 x 128-row subtiles
CHUNK = CHUNK_S * 128
CHUNKS_PER_TILE = S_DMA // CHUNK_S  # 4
TILE_ROWS = 128 * S_DMA  # 2048

F32 = mybir.dt.float32
BF16 = mybir.dt.bfloat16
NPBF16 = ml_dtypes.bfloat16


def build_bass(rows_per_core=ROWS_PER_CORE):
    nc = bacc.Bacc("TRN2", target_bir_lowering=False, debug=False)
    n_tiles = rows_per_core // TILE_ROWS
    n_chunks = rows_per_core // CHUNK

    x_ext = nc.declare_dram_parameter("x", [rows_per_core, D_IN], BF16, isOutput=False)
    eps_ext = nc.declare_dram_parameter(
        "eps", [rows_per_core, D_OUT], BF16, isOutput=False
    )
    We_ext = nc.declare_dram_parameter("W_emb", [D_IN, D_H], F32, isOutput=False)
    be_ext = nc.declare_dram_parameter("b_emb", [D_H], F32, isOutput=False)
    Wm_ext = nc.declare_dram_parameter("W_mean", [D_H, D_OUT], F32, isOutput=False)
    bm_ext = nc.declare_dram_parameter("b_mean", [D_OUT], F32, isOutput=False)
    Wl_ext = nc.declare_dram_parameter("W_logvar", [D_H, D_OUT], F32, isOutput=False)
    bl_ext = nc.declare_dram_parameter("b_logvar", [D_OUT], F32, isOutput=False)
    out_ext = nc.declare_dram_parameter(
        "out", [3, rows_per_core, D_OUT], BF16, isOutput=True
    )

    # row = t*TILE_ROWS + p*S_DMA + s ; per-partition contiguous run = s*d
    xv = x_ext.rearrange("(t p s) d -> t p (s d)", p=128, s=S_DMA)
    ev = eps_ext.rearrange("(t p s) d -> t p (s d)", p=128, s=S_DMA)
    ov = out_ext.rearrange("c (t p s) d -> t p c s d", p=128, s=S_DMA)

    with tile.TileContext(nc) as tc, ExitStack() as ctx:
        const = ctx.enter_context(tc.tile_pool(name="const", bufs=1))
        xin = ctx.enter_context(tc.tile_pool(name="xin", bufs=3))
        epool = ctx.enter_context(tc.tile_pool(name="eps", bufs=4))
        xTp = ctx.enter_context(tc.tile_pool(name="xT", bufs=3))
        hTp = ctx.enter_context(tc.tile_pool(name="hTs", bufs=3))
        stdp = ctx.enter_context(tc.tile_pool(name="std", bufs=2))
        sep = ctx.enter_context(tc.tile_pool(name="se", bufs=2))
        outs = ctx.enter_context(tc.tile_pool(name="outs", bufs=2))
        psA = ctx.enter_context(tc.tile_pool(name="psA", bufs=2, space="PSUM"))
        psB0 = ctx.enter_context(tc.tile_pool(name="psB0", bufs=1, space="PSUM"))
        psB1 = ctx.enter_context(tc.tile_pool(name="psB1", bufs=1, space="PSUM"))
        psC = ctx.enter_context(tc.tile_pool(name="psC", bufs=2, space="PSUM"))

        # --- constants / weights (loaded once) ---
        ident = const.tile([128, 128], BF16)
        masks.make_identity(nc, ident[:])

        We_sb = const.tile([128, D_H], BF16)
        nc.gpsimd.dma_start(We_sb[:], We_ext[:])
        # combined [W_mean | W_logvar]: [k-chunk partition, k, 2*D_OUT]
        Wml_sb = const.tile([128, 2, 2 * D_OUT], BF16)
        nc.gpsimd.dma_start(
            Wml_sb[:, :, 0:D_OUT], Wm_ext.rearrange("(c p) d -> p c d", p=128)
        )
        nc.gpsimd.dma_start(
            Wml_sb[:, :, D_OUT : 2 * D_OUT],
            Wl_ext.rearrange("(c p) d -> p c d", p=128),
        )

        be_sb = const.tile([128, 2], F32)
        nc.sync.dma_start(be_sb[:], be_ext.rearrange("(c p) -> p c", p=128))

        # broadcast b_mean/b_logvar across partitions via one-time K=1
        # matmuls: [128,CHUNK] = ones[1,128].T @ bias_rep[1,CHUNK]
        ones_sb = const.tile([1, 128], F32)
        nc.vector.memset(ones_sb[:], 1.0)
        bm_rep = const.tile([1, CHUNK], F32)
        bl_rep = const.tile([1, CHUNK], F32)
        for s in range(CHUNK_S):
            nc.sync.dma_start(
                bm_rep[0:1, s * D_OUT : (s + 1) * D_OUT],
                bm_ext.rearrange("(o d) -> o d", o=1),
            )
            nc.sync.dma_start(
                bl_rep[0:1, s * D_OUT : (s + 1) * D_OUT],
                bl_ext.rearrange("(o d) -> o d", o=1),
            )
        bm_bc = const.tile([128, CHUNK_S, D_OUT], F32)
        bl_bc = const.tile([128, CHUNK_S, D_OUT], F32)
        binit_ps = psC.tile([128, CHUNK_S, 2 * D_OUT], F32, tag="ml")
        nc.tensor.matmul(
            binit_ps[:].rearrange("p s d -> p (s d)")[:, 0:CHUNK],
            ones_sb[:],
            bm_rep[:],
            start=True,
            stop=True,
            skip_group_check=True,
        )
        nc.vector.tensor_copy(
            bm_bc[:].rearrange("p s d -> p (s d)"),
            binit_ps[:].rearrange("p s d -> p (s d)")[:, 0:CHUNK],
        )
        binit2_ps = psC.tile([128, CHUNK_S, 2 * D_OUT], F32, tag="ml")
        nc.tensor.matmul(
            binit2_ps[:].rearrange("p s d -> p (s d)")[:, 0:CHUNK],
            ones_sb[:],
            bl_rep[:],
            start=True,
            stop=True,
            skip_group_check=True,
        )
        nc.vector.tensor_copy(
            bl_bc[:].rearrange("p s d -> p (s d)"),
            binit2_ps[:].rearrange("p s d -> p (s d)")[:, 0:CHUNK],
        )

        # --- pipelined main loop ---
        x_tiles = {}
        eps_tiles = {}
        xT_sbs = {}
        hT_sbs = {}
        ml_pss = {}
        out_sbs = {}
        std_sbs = {}

        def fetch(t):
            if t >= n_tiles:
                return
            x_sb = xin.tile([128, S_DMA * D_IN], BF16, tag="x")
            nc.sync.dma_start(x_sb[:], xv[t])
            e_sb = epool.tile([128, S_DMA * D_OUT], BF16, tag="eps")
            nc.sync.dma_start(e_sb[:], ev[t])
            x_tiles[t] = x_sb
            eps_tiles[t] = e_sb

        fetch(0)
        fetch(1)

        for c in range(n_chunks + 5):
            # ---- stage E part 1: exp for chunk c-4 (inputs ready since
            # last iteration -> keeps ACT dense from the iteration start) ----
            g = c - 4
            if 0 <= g < n_chunks:
                t_g, j_g = divmod(g, CHUNKS_PER_TILE)
                out_sb_g = out_sbs[t_g]
                ssl_g = slice(j_g * CHUNK_S, (j_g + 1) * CHUNK_S)
                std_sb = stdp.tile([128, CHUNK_S, D_OUT], BF16, tag="std")
                nc.scalar.activation(
                    std_sb[:],
                    out_sb_g[:, 2, ssl_g, :],
                    mybir.ActivationFunctionType.Exp,
                    scale=0.5,
                )
                std_sbs[g] = std_sb

            # ---- stage D: bias adds for chunk c-3 (L2 done last iter,
            # so these are ready first on DVE) ----
            f = c - 3
            if 0 <= f < n_chunks:
                t_f, j_f = divmod(f, CHUNKS_PER_TILE)
                if j_f == 0:
                    out_sbs[t_f] = outs.tile(
                        [128, 3, S_DMA, D_OUT], BF16, tag="o", name="out_sb"
                    )
                out_sb = out_sbs[t_f]
                ml_ps = ml_pss.pop(f)
                ssl = slice(j_f * CHUNK_S, (j_f + 1) * CHUNK_S)
                nc.vector.tensor_add(
                    out_sb[:, 1, ssl, :], ml_ps[:, :, 0:D_OUT], bm_bc[:]
                )
                nc.vector.tensor_add(
                    out_sb[:, 2, ssl, :], ml_ps[:, :, D_OUT : 2 * D_OUT], bl_bc[:]
                )

            # ---- stage A: transpose chunk c ----
            if c < n_chunks:
                t, j = divmod(c, CHUNKS_PER_TILE)
                if j == 0:
                    fetch(t + 2)
                x_sb = x_tiles[t]
                xT_ps = psA.tile([128, CHUNK], BF16, tag="xT")
                for q in range(CHUNK_S):
                    s = j * CHUNK_S + q
                    nc.tensor.transpose(
                        xT_ps[:, q * 128 : (q + 1) * 128],
                        x_sb[:, s * D_IN : (s + 1) * D_IN],
                        ident[:],
                    )
                xT_sb = xTp.tile([128, CHUNK], BF16, tag="xTs")
                nc.vector.tensor_copy(xT_sb[:], xT_ps[:])
                xT_sbs[c] = xT_sb
                if j == CHUNKS_PER_TILE - 1:
                    del x_tiles[t]

            # ---- stage B: L1 + relu for chunk c-1 ----
            d = c - 1
            if 0 <= d < n_chunks:
                xT_sb = xT_sbs.pop(d)
                hT_ps0 = psB0.tile([128, CHUNK], F32, tag="hT0")
                hT_ps1 = psB1.tile([128, CHUNK], F32, tag="hT1")
                nc.tensor.matmul(
                    hT_ps0[:], We_sb[:, 0:128], xT_sb[:], start=True, stop=True
                )
                nc.tensor.matmul(
                    hT_ps1[:], We_sb[:, 128:256], xT_sb[:], start=True, stop=True
                )
                hT_sb0 = hTp.tile([128, CHUNK], BF16, tag="h0")
                hT_sb1 = hTp.tile([128, CHUNK], BF16, tag="h1")
                nc.scalar.activation(
                    hT_sb0[:],
                    hT_ps0[:],
                    mybir.ActivationFunctionType.Relu,
                    bias=be_sb[:, 0:1],
                )
                nc.scalar.activation(
                    hT_sb1[:],
                    hT_ps1[:],
                    mybir.ActivationFunctionType.Relu,
                    bias=be_sb[:, 1:2],
                )
                hT_sbs[d] = (hT_sb0, hT_sb1)

            # ---- stage C: L2 for chunk c-2 ----
            e = c - 2
            if 0 <= e < n_chunks:
                hT_sb0, hT_sb1 = hT_sbs.pop(e)
                ml_ps = psC.tile([128, CHUNK_S, 2 * D_OUT], F32, tag="ml")
                for s in range(CHUNK_S):
                    sl = slice(s * 128, (s + 1) * 128)
                    for k, hT_sbk in ((0, hT_sb0), (1, hT_sb1)):
                        nc.tensor.matmul(
                            ml_ps[:, s, :],
                            hT_sbk[:, sl],
                            Wml_sb[:, k, :],
                            start=(k == 0),
                            stop=(k == 1),
                        )
                ml_pss[e] = ml_ps

            # ---- stage E part 2: sample / store for chunk c-4 ----
            if 0 <= g < n_chunks:
                t_g, j_g = divmod(g, CHUNKS_PER_TILE)
                out_sb = out_sbs[t_g]
                ssl = slice(j_g * CHUNK_S, (j_g + 1) * CHUNK_S)
                std_sb = std_sbs.pop(g)
                e_sb = eps_tiles[t_g]
                se_sb = sep.tile([128, CHUNK_S, D_OUT], BF16, tag="se")
                nc.vector.tensor_mul(
                    se_sb[:],
                    std_sb[:],
                    e_sb[:, j_g * CHUNK : (j_g + 1) * CHUNK].rearrange(
                        "p (s d) -> p s d", s=CHUNK_S
                    ),
                )
                nc.gpsimd.tensor_add(
                    out_sb[:, 0, ssl, :], out_sb[:, 1, ssl, :], se_sb[:]
                )
                if j_g == CHUNKS_PER_TILE - 1:
                    nc.gpsimd.dma_start(ov[t_g], out_sb[:])
                    del out_sbs[t_g]
                    del eps_tiles[t_g]

    nc.finalize()
    return nc


_NC_CACHE = None


def _get_nc():
    global _NC_CACHE
    if _NC_CACHE is None:
        _NC_CACHE = build_bass()
    return _NC_CACHE


def _run(inputs, trace=False, **kw):
    nc = _get_nc()
    xs = np.ascontiguousarray(np.asarray(inputs["x"])).astype(NPBF16)
    es = np.ascontiguousarray(np.asarray(inputs["eps"])).astype(NPBF16)
    weights = {
        k: np.ascontiguousarray(np.asarray(inputs[k], dtype=np.float32))
        for k in ("W_emb", "b_emb", "W_mean", "b_mean", "W_logvar", "b_logvar")
    }
    in_maps = []
    for c in range(N_CORES):
        sl = slice(c * ROWS_PER_CORE, (c + 1) * ROWS_PER_CORE)
        in_maps.append({"x": xs[sl], "eps": es[sl], **weights})
    res = run_bass_kernel_spmd(nc, in_maps, list(range(N_CORES)), trace=trace, **kw)
    z = np.concatenate(
        [res.results[c]["out"][0] for c in range(N_CORES)], axis=0
    ).astype(np.float32)
    mean = np.concatenate(
        [res.results[c]["out"][1] for c in range(N_CORES)], axis=0
    ).astype(np.float32)
    lv = np.concatenate(
        [res.results[c]["out"][2] for c in range(N_CORES)], axis=0
    ).astype(np.float32)
    return (z, mean, lv), res


def kernel(**inputs):
    out, _ = _run(inputs, trace=False)
    return out


if __name__ == "__main__":
    rng = np.random.default_rng(0)
    demo = {
        "x": rng.standard_normal((B, D_IN), dtype=np.float32),
        "eps": rng.standard_normal((B, D_OUT), dtype=np.float32),
        "W_emb": rng.standard_normal((D_IN, D_H), dtype=np.float32) * 0.088,
        "b_emb": rng.standard_normal((D_H,), dtype=np.float32) * 0.05,
        "W_mean": rng.standard_normal((D_H, D_OUT), dtype=np.float32) * 0.06,
        "b_mean": rng.standard_normal((D_OUT,), dtype=np.float32) * 0.03,
        "W_logvar": rng.standard_normal((D_H, D_OUT), dtype=np.float32) * 0.06,
        "b_logvar": rng.standard_normal((D_OUT,), dtype=np.float32) * 0.03,
    }
    z, m, l = kernel(**demo)
    print("shapes", z.shape, m.shape, l.shape)
